# revision 23
# baseline (speedup 1.0000x reference)
"""Trainium2 Bass kernel for the LogRatio loss (nn_LogRatio_14104672600201).

v3 — fp8 DoubleRow reductions with a (y, y2)-interleaved k-pair dim.

Math: every masked reduction over logsim[j, l] = log((X @ X.T)[j, l]) has a
mask depending on j only through g_j = targets[j] in [0, 64). The y^2-weighted
sums appear only LINEARLY in the loss, so they fold into

  Wz[l, g] = h0(g) * P[l, g] + (c(g) - 1) * W0[l, g]     (weights y^2)
  Wy[l, g] = 0.2 * (c(g) - 1) * W1[l, g]                 (weights y)

Device reductions per j (selected at g = t_j by the one-hot mask):
  yP, yW0 = [P | W0].T @ y        L = Wy.T @ y + Wz.T @ y2

fp8 DoubleRow trick: a DR matmul contracts over (partition, k-pair). We lay
y and y2 for each l-tile interleaved in one fp8 tile u[128, 2(i), 2(h), 512]:

  pass2 (per tile):  lhsT = [Wy_i0 | Wz_i1]/16 [128,2,64], rhs = u[:,:,h,:]
     -> out[64, 512] accumulates Wy.T y + (Wz/16).T y2 in ONE plain matmul
  x1a (per PAIR):    lhsT = [P|W0] of both tiles [128,2,128], rhs = u[:,0,:,:]
     -> k-pair dim = the two l-tiles; ONE matmul reduces 256 l rows

PE per pair of tiles: 2 bf16 sims (2x512cyc) + x1a_d (512cyc) + 2 pass2
(2x512cyc at 2 elem/cycle) -- no tile_position pairs, no column-split taxes.
The scalar engine does 16 N=1024 Ln activations (fp8 out); DVE squares fp8.

Tail on-chip: mask-mul + PE collapse to 3 rows, 8KB DMA out, host float64.
"""

import numpy as np
import ml_dtypes

N, D, KK, C = 4096, 128, 4, 64
NCORES = 8
JSH = N // NCORES          # 512 j rows per core
LT = N // 128              # 32 l-tiles
PAIRS = LT // 2            # 16 sim/reduce pairs
EPS = 1e-6
OMEGA = 0.1
KSC = float(np.float32(np.exp(-3.5)))        # Ln input scale (exactly f32)
SHIFT = float(-np.log(np.float64(KSC)))      # effective shift s = -ln(KSC)
WSC = 32.0                                   # Wy/Wz fp8 range scale (e4m3 max 240)

_CACHE = {}


def _build_nc():
    import concourse.bass as bass
    import concourse.bacc as bacc
    import concourse.mybir as mybir
    import concourse.tile as tile
    from contextlib import ExitStack

    f32 = mybir.dt.float32
    bf16 = mybir.dt.bfloat16
    fp8 = mybir.dt.float8e4
    Ln = mybir.ActivationFunctionType.Ln
    DR = mybir.MatmulPerfMode.DoubleRow

    nc = bacc.Bacc("TRN2", target_bir_lowering=False, debug=False)
    xt = nc.dram_tensor("xt", [D, N], bf16, kind="ExternalInput")
    # qa[p, pr, i, m] = [P | W0][(2*pr + i)*128 + p, m]
    qa = nc.dram_tensor("qa", [128, PAIRS, 2, 128], fp8, kind="ExternalInput")
    # qyz[p, lt, 0, m] = Wy[lt*128+p, m]/16 ; [p, lt, 1, m] = Wz[...]/16
    qyz = nc.dram_tensor("qyz", [128, LT, 2, 64], fp8, kind="ExternalInput")
    # msk[g, j] = (g % 64 == t_j) for j cols 0:512
    msk = nc.dram_tensor("msk", [128, JSH], bf16, kind="ExternalInput")
    # masked accumulators; host does the (tiny) partition sums in float64
    lout1 = nc.dram_tensor("lout1", [128, JSH], bf16, kind="ExternalOutput")
    lout2 = nc.dram_tensor("lout2", [64, JSH], bf16, kind="ExternalOutput")

    NP = 2    # sim double-tile slots (2 PSUM banks each)
    NLP = 6   # u (y/y2 fp8) SBUF slots

    with tile.TileContext(nc) as tc, ExitStack() as ctx:
        cpool = ctx.enter_context(tc.tile_pool(name="const", bufs=1))
        mpool = ctx.enter_context(tc.tile_pool(name="mpool", bufs=1))
        px = ctx.enter_context(tc.tile_pool(name="px", bufs=1, space="PSUM"))

        # ---- DMAs first: earliest descriptor-gen slots on the Sync queue.
        xt_sb = cpool.tile([D, N], bf16, tag="xt")
        nc.sync.dma_start(xt_sb[:, 0:512], xt[:, 0:512])
        qa_sb = cpool.tile([128, PAIRS, 2, 128], fp8, tag="qa")
        nc.sync.dma_start(qa_sb[:, 0:4, :, :], qa[:, 0:4, :, :])
        nc.sync.dma_start(xt_sb[:, 512:2048], xt[:, 512:2048])
        qyz_sb = cpool.tile([128, LT, 2, 64], fp8, tag="qyz")
        nc.sync.dma_start(qyz_sb[:, 0:8, :, :], qyz[:, 0:8, :, :])
        nc.sync.dma_start(qa_sb[:, 4:16, :, :], qa[:, 4:16, :, :])
        nc.sync.dma_start(xt_sb[:, 2048:4096], xt[:, 2048:4096])
        nc.sync.dma_start(qyz_sb[:, 8:32, :, :], qyz[:, 8:32, :, :])
        msk_sb = cpool.tile([128, JSH], bf16, tag="msk")
        nc.sync.dma_start(msk_sb[:], msk[:])

        # ---- PE warm-up: DVE memset feeds back-to-back dummy matmuls so
        # the HAM clock-gate opens (1.2 -> 2.4 GHz) and the PE stays busy
        # until the first xt chunk's 16-engine DMA semaphore fires (the
        # straggler engine can be ~3us late). Output bank is never read.
        # gpsimd's preamble completes earliest, so the warm-up dummies can
        # start ~1.2us sooner than behind the Vector preamble chain
        scr = cpool.tile([128, JSH], bf16, tag="scr")
        nc.gpsimd.memset(scr[:], 0.0)
        warm = px.tile([128, JSH], f32, tag="warm", name="warm")
        for _ in range(9):
            nc.tensor.matmul(
                warm[:], scr[:, 0:128], scr[:], start=True, stop=True
            )

        # accumulators (one PSUM bank each, held across the whole loop)
        x1a = px.tile([128, JSH], f32, tag="x1a")
        xL = px.tile([64, JSH], f32, tag="xL")

        simds = [
            px.tile([128, 2, 512], f32, tag=f"simd{i}", name=f"simd{i}")
            for i in range(NP)
        ]
        uus = [
            cpool.tile([128, 2, 2, 512], fp8, tag=f"uu{i}", name=f"uu{i}")
            for i in range(NLP)
        ]

        mov = xt_sb[:, 0:JSH]

        def sim_pair(p):
            sd = simds[p % NP]
            nc.tensor.matmul(
                sd[:, 0, :], xt_sb[:, bass.ts(2 * p, 128)], mov,
                start=True, stop=True,
            )
            nc.tensor.matmul(
                sd[:, 1, :], xt_sb[:, bass.ts(2 * p + 1, 128)], mov,
                start=True, stop=True,
            )
            u = uus[p % NLP]
            nc.scalar.activation(u[:, 0, :, :], sd[:, :, :], Ln, scale=KSC)
            nc.vector.tensor_mul(u[:, 1, :, :], u[:, 0, :, :], u[:, 0, :, :])

        for p in range(NP):
            sim_pair(p)
        for pr in range(PAIRS):
            if pr + NP < PAIRS:
                sim_pair(pr + NP)
            if pr in (0, 1):
                # dependency-free keepalive for any residual DMA-stall idle
                nc.tensor.matmul(
                    warm[:], scr[:, 0:128], scr[:], start=True, stop=True
                )
            u = uus[pr % NLP]
            nc.tensor.matmul(
                x1a[:], qa_sb[:, pr, :, :], u[:, 0, :, :],
                start=pr == 0, stop=pr == PAIRS - 1, perf_mode=DR,
            )
            for h in (0, 1):
                lt = 2 * pr + h
                nc.tensor.matmul(
                    xL[:], qyz_sb[:, lt, :, :], u[:, :, h, :],
                    start=lt == 0, stop=lt == LT - 1, perf_mode=DR,
                )

        # ---- tail: mask-mul and DMA the masked tiles straight out; the
        # DMA completion receipt hides under the fixed end-barrier churn.
        # Host does the 64/128-row partition sums in float64.
        m1 = mpool.tile([128, JSH], bf16, tag="m1", name="m1")
        nc.vector.tensor_mul(m1[:], x1a[:], msk_sb[:])
        nc.sync.dma_start(lout1[:], m1[:])
        mL = mpool.tile([64, JSH], bf16, tag="mL", name="mL")
        nc.vector.tensor_mul(mL[:], xL[:], msk_sb[0:64, :])
        nc.sync.dma_start(lout2[:], mL[:])
    nc.compile()
    return nc


def _host_tables(labels):
    lab = np.asarray(labels)
    t = lab[:, 0].astype(np.int64)
    m = np.arange(KK)
    om = np.float64(OMEGA)
    lp = np.log(np.float64(OMEGA + EPS)) - np.log(
        om ** (KK - m + 1) + np.float64(EPS)
    )
    gr = np.arange(C)
    eq = lab[None, :, :] == gr[:, None, None]          # [C, N, KK]
    nm = np.stack(
        [
            ~eq[:, :, 3],
            eq[:, :, 3] & ~eq[:, :, 2],
            eq[:, :, 2] & ~eq[:, :, 1],
            eq[:, :, 1] & ~eq[:, :, 0],
        ]
    ).astype(np.float64)                                # [KK, C, N]
    w0 = nm.sum(0)                                      # [C, N] (g, l)
    w1 = np.einsum("m,mcl->cl", lp, nm)
    w2 = np.einsum("m,mcl->cl", lp * lp, nm)
    ph = (t[:, None] == gr[None, :]).astype(np.float64)  # [N, C] = P[l, g]
    cnt = ph.sum(0)                                      # [C]
    h0, h1, h2 = w0.sum(1), w1.sum(1), w2.sum(1)         # [C]
    wy = 0.2 * (cnt[:, None] - 1.0) * w1 / WSC           # [C, N]
    wz = (h0[:, None] * ph.T + (cnt[:, None] - 1.0) * w0) / WSC
    # qam[l, m] for m in 0:128 = [P | W0]
    qam = np.concatenate([ph, w0.T], axis=1).astype(np.float32)   # [N, 128]
    # qyzm[l, i, m]: i=0 -> Wy/16, i=1 -> Wz/16
    qyzm = np.stack([wy.T, wz.T], axis=1).astype(np.float32)      # [N, 2, 64]
    return t, cnt, h0, h1, h2, qam, qyzm


def _host_prep(inputs, labels):
    x = np.asarray(inputs, dtype=np.float32)
    bf = ml_dtypes.bfloat16
    f8 = ml_dtypes.float8_e4m3
    t, cnt, h0, h1, h2, qam, qyzm = _host_tables(labels)
    gr = np.arange(C)

    xt = np.ascontiguousarray(x.T)                       # [D, N]
    in_maps = []
    for cid in range(NCORES):
        sl = slice(cid * JSH, (cid + 1) * JSH)
        # rotate the l axis so this core's own j-shard sits at columns
        # 0:JSH; the l reduction (over all 4096) is rotation-invariant as
        # long as the q tables rotate identically.
        xtc = np.roll(xt, -cid * JSH, axis=1)
        qac = np.roll(qam, -cid * JSH, axis=0)           # [N, 128]
        qyzc = np.roll(qyzm, -cid * JSH, axis=0)         # [N, 2, 64]
        # qa_sb[p, pr, i, m] = qac[(2*pr + i)*128 + p, m]
        qa_sb = np.ascontiguousarray(
            qac.reshape(PAIRS, 2, 128, 128).transpose(2, 0, 1, 3)
        )
        # qyz_sb[p, lt, i, m] = qyzc[lt*128 + p, i, m]
        qyz_sb = np.ascontiguousarray(
            qyzc.reshape(LT, 128, 2, 64).transpose(1, 0, 2, 3)
        )
        oh = (gr[:, None] == t[sl][None, :]).astype(np.float32)  # [64, 512]
        mk = np.concatenate([oh, oh], axis=0)                    # [128, 512]
        in_maps.append(
            {
                "xt": xtc.astype(bf),
                "qa": qa_sb.astype(f8),
                "qyz": qyz_sb.astype(f8),
                "msk": mk.astype(bf),
            }
        )

    tabs = {"t": t, "cnt": cnt, "h0": h0, "h1": h1, "h2": h2, "x": x}
    return in_maps, tabs


def _host_loss(res_list, tabs):
    t, cnt, h0, h1, h2 = tabs["t"], tabs["cnt"], tabs["h0"], tabs["h1"], tabs["h2"]
    x64 = tabs["x"].astype(np.float64)
    s = np.float64(SHIFT)
    loss = np.float64(0.0)
    for cid, r in enumerate(res_list):
        sl = slice(cid * JSH, (cid + 1) * JSH)
        lo1 = r["lout1"].astype(np.float64)              # [128, JSH]
        yP, yW0 = lo1[0:64].sum(0), lo1[64:128].sum(0)
        L = WSC * r["lout2"].astype(np.float64).sum(0)
        tj = t[sl]
        cg, h0j, h1j, h2j = cnt[tj], h0[tj], h1[tj], h2[tj]
        diag = np.log(np.einsum("jd,jd->j", x64[sl], x64[sl]) + EPS)
        S1 = yP + s * cg - diag
        A1 = yW0 + s * h0j + 0.1 * h1j
        lossj = (
            L
            + 2.0 * s * h0j * yP - h0j * diag * diag + s * s * cg * h0j
            + 2.0 * s * (cg - 1.0) * yW0 + s * s * (cg - 1.0) * h0j
            + 0.2 * s * (cg - 1.0) * h1j + 0.01 * (cg - 1.0) * h2j
            - 2.0 * S1 * A1
        )
        loss += lossj.sum()
    return np.array(loss, dtype=np.float32)


def _run(inputs, labels, trace=False, tmpdir=None):
    from concourse.bass_utils import run_bass_kernel_spmd

    if "nc" not in _CACHE:
        _CACHE["nc"] = _build_nc()
    in_maps, tabs = _host_prep(inputs, labels)
    res = run_bass_kernel_spmd(
        _CACHE["nc"], in_maps, core_ids=list(range(NCORES)),
        trace=trace, tmpdir=tmpdir,
    )
    return _host_loss(res.results, tabs), res


def kernel(inputs, labels):
    out, _ = _run(inputs, labels, trace=False)
    return out
